# revision 9
# baseline (speedup 1.0000x reference)
"""Bass/Trainium2 kernel for nn_Decoder (2-layer bidir-style LSTM decoder
with general attention + fc), distributed over 8 NeuronCores.

Architecture (SPMD, one uniform program; per-core behavior differs only in
input DATA):
  - 4 LSTM cells (L0f, L0b, L1f, L1b) -> cores 0..3; cores 4..7 mirror 0..3.
  - Sequential scan is chunked: CH=32 timesteps per chunk. One AllGather per
    chunk exchanges each cell's h-chunk. Layer 1 runs one chunk behind
    layer 0 (its input-to-hidden term consumes L0's h from the previous AG).
  - Per-chunk input-to-hidden precompute: ih = W_emb @ emb_chunk +
    W_in @ [h0f; h0b]_chunk (+bias); per-core zero weights make the same
    program compute the right thing for every cell.
  - Per step: gates = ih[:, t] + Whh @ h_{t-1} via PE (weights stationary,
    gates on partitions, batch=16 moving), then an 8-op ACT/DVE chain for
    the LSTM nonlinearity. Gate order is permuted to i,f,o,g on host so one
    Sigmoid covers i,f,o and one Tanh covers g.
  - dec_t = h1f + h1b accumulated from AG slots 2,3 into per-core DRAM.
  - Post phase (after scan): per-core, its 2 batch elements: enc_proj,
    masked softmax attention, context, fc, all as dense bf16 matmuls.

Numerics: bf16 weights/activations with fp32 PSUM accumulation and fp32
cell state c. Expected rel err ~1e-3..1e-2 vs the fp32 reference.
"""

import os
import sys

sys.path.insert(0, "/opt/trn_rl_repo")

import numpy as np
import ml_dtypes

import concourse.bass as bass
import concourse.mybir as mybir
import concourse.tile as tile
from concourse import bacc
from concourse.bass_utils import run_bass_kernel_spmd

# ---- problem constants (hardcoded per contract) ----
L = 2
H = 512
E = 512
B = 16
T = 512
S = 512
VOCAB = 1001
OUT = 1000

N_CORES = 8
CH = 32                       # timesteps per chunk
NCH = T // CH                 # 16 chunks
ITERS = NCH + 1               # L1 lags one chunk
COLS = CH * B                 # 512 columns per chunk (s-major, b-minor)
HC = H // 128                 # 4 H-chunks
MC = (4 * H) // 128           # 16 gate M-chunks
TC = T // 128                 # 4 T-chunks (post phase)
SC = S // 128                 # 4 S-chunks
BF = mybir.dt.bfloat16
F32 = mybir.dt.float32
I32 = mybir.dt.int32

# gate permutation: torch order i,f,g,o -> i,f,o,g  (rows of the 4H dim)
def _gate_perm():
    idx = np.arange(4 * H)
    return np.concatenate([idx[0:H], idx[H:2 * H], idx[3 * H:4 * H], idx[2 * H:3 * H]])


def _bf(x):
    return np.ascontiguousarray(np.asarray(x, dtype=np.float32)).astype(ml_dtypes.bfloat16)


def _f32(x):
    return np.ascontiguousarray(np.asarray(x, dtype=np.float32))


def build_nc(nch=NCH):
    iters = nch + 1
    t_total = nch * CH
    tc_n = t_total // 128
    nc = bacc.Bacc("TRN2", target_bir_lowering=False, debug=False, num_devices=N_CORES)

    # ---- DRAM inputs ----
    w_emb = nc.dram_tensor("w_emb", [E, 4 * H], BF, kind="ExternalInput")
    w_in = nc.dram_tensor("w_in", [2 * H, 4 * H], BF, kind="ExternalInput")
    w_hh = nc.dram_tensor("w_hh", [H, 4 * H], BF, kind="ExternalInput")
    biasp = nc.dram_tensor("biasp", [128, MC], F32, kind="ExternalInput")
    h_init = nc.dram_tensor("h_init", [128, HC, B], BF, kind="ExternalInput")
    c_init = nc.dram_tensor("c_init", [128, HC, B], F32, kind="ExternalInput")
    alpha = nc.dram_tensor("alpha", [128, 1], F32, kind="ExternalInput")
    beta = nc.dram_tensor("beta", [128, 1], F32, kind="ExternalInput")
    emb_stream = nc.dram_tensor("emb_stream", [iters, E, COLS], BF, kind="ExternalInput")
    enc_lhsT = nc.dram_tensor("enc_lhsT", [B, S, H], BF, kind="ExternalInput")
    encT_rhs = nc.dram_tensor("encT_rhs", [B, H, S], BF, kind="ExternalInput")
    w_attT = nc.dram_tensor("w_attT", [E, H], BF, kind="ExternalInput")
    b_att_in = nc.dram_tensor("b_att_in", [128, HC], F32, kind="ExternalInput")
    mask_row = nc.dram_tensor("mask_row", [B, 1, S], BF, kind="ExternalInput")
    valid_in = nc.dram_tensor("valid_in", [B, 128, tc_n], F32, kind="ExternalInput")
    w_fcT = nc.dram_tensor("w_fcT", [2 * H, OUT], BF, kind="ExternalInput")
    b_fc_row = nc.dram_tensor("b_fc_row", [1, OUT], BF, kind="ExternalInput")
    out_d = nc.dram_tensor("out", [B, t_total, OUT], F32, kind="ExternalOutput")

    # ---- DRAM internals ----
    ag_out = nc.dram_tensor("ag_out", [N_CORES, H, COLS], BF, addr_space="Shared")
    dec_dram = nc.dram_tensor("dec_dram", [B, nch, 128, HC, CH], BF)

    groups = [list(range(N_CORES))]

    with tile.TileContext(nc) as tc:
        with (
            tc.tile_pool(name="wpool", bufs=1) as wpool,
            tc.tile_pool(name="spool", bufs=2) as spool,
            tc.tile_pool(name="steppool", bufs=3) as steppool,
            tc.tile_pool(name="pg", bufs=2, space="PSUM") as pg,
            tc.tile_pool(name="pih", bufs=2, space="PSUM") as pih,
            tc.tile_pool(name="ppost", bufs=2, space="PSUM") as ppost,
            tc.tile_pool(name="dpool", bufs=2, space="DRAM") as dpool,
        ):
            # ---- persistent SBUF ----
            wemb_sb = wpool.tile([128, HC, 4 * H], BF, tag="wemb")
            nc.sync.dma_start(wemb_sb[:], w_emb.rearrange("(k p) m -> p k m", p=128))
            win_sb = wpool.tile([128, 2 * HC, 4 * H], BF, tag="win")
            nc.sync.dma_start(win_sb[:], w_in.rearrange("(k p) m -> p k m", p=128))
            whh_sb = wpool.tile([128, HC, 4 * H], BF, tag="whh")
            nc.sync.dma_start(whh_sb[:], w_hh.rearrange("(k p) m -> p k m", p=128))
            biasp_sb = wpool.tile([128, MC], F32, tag="biasp")
            nc.sync.dma_start(biasp_sb[:], biasp[:])
            hinit_sb = wpool.tile([128, HC, B], BF, tag="hinit")
            nc.sync.dma_start(hinit_sb[:], h_init[:])
            cinit_sb = wpool.tile([128, HC, B], F32, tag="cinit")
            nc.sync.dma_start(cinit_sb[:], c_init[:])
            alpha_sb = wpool.tile([128, 1], F32, tag="alpha")
            nc.sync.dma_start(alpha_sb[:], alpha[:])
            beta_sb = wpool.tile([128, 1], F32, tag="beta")
            nc.sync.dma_start(beta_sb[:], beta[:])

            # zero-fill ag_in once, prologue AllGather -> ag_out defined zeros
            zer = spool.tile([128, HC, COLS], BF, tag="zer", bufs=1)
            nc.vector.memset(zer[:], 0.0)
            ag_in0 = dpool.tile([H, COLS], BF, tag="ag_in")
            nc.sync.dma_start(ag_in0.rearrange("(k p) c -> p k c", p=128), zer[:])
            nc.gpsimd.collective_compute(
                "AllGather", mybir.AluOpType.bypass, replica_groups=groups,
                ins=[ag_in0.opt()], outs=[ag_out[:].opt()],
            )

            accum_prev = None
            c_cur = None
            for k in range(iters):
                # ---------- per-iteration stream tiles ----------
                emb_t = spool.tile([128, HC, COLS], BF, tag="emb_t")
                nc.sync.dma_start(
                    emb_t[:], emb_stream[k].rearrange("(k p) c -> p k c", p=128)
                )
                xh_t = spool.tile([128, 2 * HC, COLS], BF, tag="xh_t")
                nc.sync.dma_start(
                    xh_t[:],
                    ag_out[0:2].rearrange("s (k p) c -> p (s k) c", p=128),
                )

                # ---------- ih precompute for this chunk ----------
                ih = spool.tile([128, MC, COLS], BF, tag="ih", bufs=1)
                for m in range(MC):
                    ps = pih.tile([128, COLS], F32, tag="ihps")
                    for kk in range(HC):
                        nc.tensor.matmul(
                            ps[:], wemb_sb[:, kk, m * 128:(m + 1) * 128],
                            emb_t[:, kk, :], start=(kk == 0), stop=False,
                        )
                    for kk in range(2 * HC):
                        nc.tensor.matmul(
                            ps[:], win_sb[:, kk, m * 128:(m + 1) * 128],
                            xh_t[:, kk, :], start=False, stop=(kk == 2 * HC - 1),
                        )
                    nc.vector.tensor_scalar_add(ih[:, m, :], ps[:], biasp_sb[:, m:m + 1])

                # ---------- state carry / blend ----------
                accum = spool.tile([128, HC, (CH + 1) * B], BF, tag="accum")
                if k == 0:
                    nc.vector.tensor_copy(accum[:, :, 0:B], hinit_sb[:])
                    c_new0 = steppool.tile([128, HC, B], F32, tag="c")
                    nc.vector.tensor_copy(c_new0[:], cinit_sb[:])
                    c_cur = c_new0
                elif k == 1:
                    # L1 cores reset to init (alpha=0,beta=1); L0 keep (1,0)
                    t1 = steppool.tile([128, HC, B], F32, tag="blend")
                    nc.vector.tensor_scalar_mul(t1[:], accum_prev[:, :, CH * B:], alpha_sb[:, 0:1])
                    t2 = steppool.tile([128, HC, B], F32, tag="blend")
                    nc.vector.tensor_scalar_mul(t2[:], hinit_sb[:], beta_sb[:, 0:1])
                    nc.vector.tensor_add(accum[:, :, 0:B], t1[:], t2[:])
                    t3 = steppool.tile([128, HC, B], F32, tag="blend")
                    nc.vector.tensor_scalar_mul(t3[:], c_cur[:], alpha_sb[:, 0:1])
                    t4 = steppool.tile([128, HC, B], F32, tag="blend")
                    nc.vector.tensor_scalar_mul(t4[:], cinit_sb[:], beta_sb[:, 0:1])
                    c_new1 = steppool.tile([128, HC, B], F32, tag="c")
                    nc.vector.tensor_add(c_new1[:], t3[:], t4[:])
                    c_cur = c_new1
                else:
                    nc.vector.tensor_copy(accum[:, :, 0:B], accum_prev[:, :, CH * B:])

                # ---------- CH recurrence steps ----------
                for s in range(CH):
                    g_ps = pg.tile([128, MC, B], F32, tag="g")
                    for m in range(MC):
                        for kk in range(HC):
                            nc.tensor.matmul(
                                g_ps[:, m, :],
                                whh_sb[:, kk, m * 128:(m + 1) * 128],
                                accum[:, kk, s * B:(s + 1) * B],
                                start=(kk == 0), stop=(kk == HC - 1),
                            )
                    g_sb = steppool.tile([128, MC, B], F32, tag="g_sb")
                    nc.vector.tensor_add(g_sb[:], g_ps[:], ih[:, :, s * B:(s + 1) * B])
                    sig = steppool.tile([128, 12, B], F32, tag="sig")
                    nc.scalar.activation(sig[:], g_sb[:, 0:12, :], mybir.ActivationFunctionType.Sigmoid)
                    tg = steppool.tile([128, HC, B], F32, tag="tg")
                    nc.scalar.activation(tg[:], g_sb[:, 12:16, :], mybir.ActivationFunctionType.Tanh)
                    m1 = steppool.tile([128, HC, B], F32, tag="m1")
                    nc.vector.tensor_mul(m1[:], sig[:, 4:8, :], c_cur[:])
                    m2 = steppool.tile([128, HC, B], F32, tag="m2")
                    nc.vector.tensor_mul(m2[:], sig[:, 0:4, :], tg[:])
                    c_new = steppool.tile([128, HC, B], F32, tag="c")
                    nc.vector.tensor_add(c_new[:], m1[:], m2[:])
                    tc_t = steppool.tile([128, HC, B], F32, tag="tc")
                    nc.scalar.activation(tc_t[:], c_new[:], mybir.ActivationFunctionType.Tanh)
                    nc.vector.tensor_mul(accum[:, :, (s + 1) * B:(s + 2) * B], sig[:, 8:12, :], tc_t[:])
                    c_cur = c_new

                # ---------- exchange ----------
                ag_in = dpool.tile([H, COLS], BF, tag="ag_in")
                nc.sync.dma_start(
                    ag_in.rearrange("(k p) c -> p k c", p=128), accum[:, :, B:]
                )
                nc.gpsimd.collective_compute(
                    "AllGather", mybir.AluOpType.bypass, replica_groups=groups,
                    ins=[ag_in.opt()], outs=[ag_out[:].opt()],
                )

                # ---------- dec extraction (chunk k-1) ----------
                if k >= 1:
                    s23 = spool.tile([128, 2, HC, COLS], BF, tag="s23", bufs=1)
                    nc.sync.dma_start(
                        s23[:], ag_out[2:4].rearrange("s (k p) c -> p s k c", p=128)
                    )
                    dsum = spool.tile([128, HC, CH, B], BF, tag="dsum", bufs=1)
                    nc.vector.tensor_add(
                        dsum[:],
                        s23[:, 0].rearrange("p k (s b) -> p k s b", b=B),
                        s23[:, 1].rearrange("p k (s b) -> p k s b", b=B),
                    )
                    for lb in range(B):
                        nc.sync.dma_start(
                            dec_dram[lb, k - 1], dsum[:, :, :, lb].opt()
                        )
                accum_prev = accum

            # ================= post phase =================
            identity = wpool.tile([128, 128], BF, tag="ident")
            from concourse.masks import make_identity
            make_identity(nc, identity[:])
            ones1 = wpool.tile([1, 128], BF, tag="ones1")
            nc.vector.memset(ones1[:], 1.0)

            wattT_sb = wpool.tile([128, HC, H], BF, tag="wattT")
            nc.sync.dma_start(wattT_sb[:], w_attT.rearrange("(k p) m -> p k m", p=128))
            batt_sb = wpool.tile([128, HC], F32, tag="batt")
            nc.sync.dma_start(batt_sb[:], b_att_in[:])
            wfc_sb = wpool.tile([128, 2 * HC, OUT], BF, tag="wfc")
            nc.sync.dma_start(wfc_sb[:], w_fcT.rearrange("(k p) m -> p k m", p=128))
            bfc_sb = wpool.tile([1, OUT], BF, tag="bfc")
            nc.sync.dma_start(bfc_sb[:], b_fc_row[:])

            for lb in range(B):
                encT_sb = spool.tile([128, HC, S], BF, tag="encT", bufs=1)
                nc.sync.dma_start(
                    encT_sb[:], encT_rhs[lb].rearrange("(k p) s -> p k s", p=128)
                )
                enc_sb = spool.tile([128, SC, H], BF, tag="enc", bufs=1)
                nc.sync.dma_start(
                    enc_sb[:], enc_lhsT[lb].rearrange("(k p) h -> p k h", p=128)
                )
                mask_sb = spool.tile([1, S], BF, tag="mask", bufs=1)
                nc.sync.dma_start(mask_sb[:], mask_row[lb])
                valid_sb = spool.tile([128, tc_n], F32, tag="valid", bufs=1)
                nc.sync.dma_start(valid_sb[:], valid_in[lb])
                dec_sb = spool.tile([128, HC, t_total], BF, tag="dec_sb", bufs=1)
                for kc in range(nch):
                    nc.sync.dma_start(
                        dec_sb[:, :, kc * CH:(kc + 1) * CH], dec_dram[lb, kc]
                    )

                # enc_projT [H, S]
                epT = spool.tile([128, HC, S], BF, tag="epT", bufs=1)
                for m in range(HC):
                    pp = ppost.tile([128, S], F32, tag="pp")
                    for e in range(HC):
                        nc.tensor.matmul(
                            pp[:], wattT_sb[:, e, m * 128:(m + 1) * 128],
                            encT_sb[:, e, :], start=(e == 0), stop=(e == HC - 1),
                        )
                    nc.vector.tensor_scalar_add(epT[:, m, :], pp[:], batt_sb[:, m:m + 1])

                attT = spool.tile([128, SC, t_total], BF, tag="attT", bufs=1)
                for t in range(tc_n):
                    sp = ppost.tile([128, S], F32, tag="pp")
                    for hk in range(HC):
                        nc.tensor.matmul(
                            sp[:], dec_sb[:, hk, t * 128:(t + 1) * 128],
                            epT[:, hk, :], start=(hk == 0), stop=False,
                        )
                    nc.tensor.matmul(
                        sp[:], ones1[:], mask_sb[:], start=False, stop=True,
                    )
                    mx = steppool.tile([128, 1], F32, tag="mx")
                    nc.vector.reduce_max(mx[:], sp[:], axis=mybir.AxisListType.X)
                    negmax = steppool.tile([128, 1], F32, tag="negmax")
                    nc.scalar.mul(negmax[:], mx[:], -1.0)
                    att = steppool.tile([128, S], BF, tag="att")
                    sumexp = steppool.tile([128, 1], F32, tag="sumexp")
                    nc.scalar.activation(
                        att[:], sp[:], mybir.ActivationFunctionType.Exp,
                        bias=negmax[:], accum_out=sumexp[:],
                    )
                    recip = steppool.tile([128, 1], F32, tag="recip")
                    nc.vector.reciprocal(recip[:], sumexp[:])
                    attn = steppool.tile([128, S], BF, tag="attn")
                    nc.vector.tensor_scalar_mul(attn[:], att[:], recip[:])
                    for sk in range(SC):
                        tp = ppost.tile([128, 128], BF, tag="ptr")
                        nc.tensor.transpose(tp[:], attn[:, sk * 128:(sk + 1) * 128], identity[:])
                        nc.vector.tensor_copy(attT[:, sk, t * 128:(t + 1) * 128], tp[:])

                ctxT = spool.tile([128, HC, t_total], BF, tag="ctxT", bufs=1)
                for m in range(HC):
                    cp = ppost.tile([128, t_total], F32, tag="pp")
                    for sk in range(SC):
                        nc.tensor.matmul(
                            cp[:], enc_sb[:, sk, m * 128:(m + 1) * 128],
                            attT[:, sk, :], start=(sk == 0), stop=(sk == SC - 1),
                        )
                    nc.vector.tensor_copy(ctxT[:, m, :], cp[:])

                for t in range(tc_n):
                    for nhalf in range(2):
                        ncols = OUT // 2
                        fp = ppost.tile([128, ncols], F32, tag="pp")
                        for kk in range(2 * HC):
                            lhs = (dec_sb[:, kk, t * 128:(t + 1) * 128] if kk < HC
                                   else ctxT[:, kk - HC, t * 128:(t + 1) * 128])
                            nc.tensor.matmul(
                                fp[:], lhs, wfc_sb[:, kk, nhalf * ncols:(nhalf + 1) * ncols],
                                start=(kk == 0), stop=False,
                            )
                        nc.tensor.matmul(
                            fp[:], ones1[:], bfc_sb[:, nhalf * ncols:(nhalf + 1) * ncols],
                            start=False, stop=True,
                        )
                        osb = steppool.tile([128, ncols], F32, tag="osb")
                        nc.vector.tensor_scalar_mul(osb[:], fp[:], valid_sb[:, t:t + 1])
                        nc.sync.dma_start(
                            out_d[lb, t * 128:(t + 1) * 128, nhalf * ncols:(nhalf + 1) * ncols],
                            osb[:],
                        )

    nc.compile()
    return nc


# ---------------- host-side preparation ----------------

def _prep_inputs(inputs, nch=NCH):
    iters = nch + 1
    t_total = nch * CH
    tc_n = t_total // 128
    perm = _gate_perm()

    trg = np.asarray(inputs["trg_inputs"]).astype(np.int64)
    trg_len = np.asarray(inputs["trg_len"]).astype(np.int64)
    enc = _f32(inputs["encoder_outputs"])
    h0 = _f32(inputs["h0"]).reshape(L, 2, B, H)
    c0 = _f32(inputs["c0"]).reshape(L, 2, B, H)
    embed = _f32(inputs["embed"])
    W_ih0 = _f32(inputs["W_ih0"])          # [2, 4H, E]
    W_ih1 = _f32(inputs["W_ih1"])[0]       # [2, 4H, 2H]
    W_hh = _f32(inputs["W_hh"])            # [L, 2, 4H, H]
    b_ih = _f32(inputs["b_ih"])            # [L, 2, 4H]
    b_hh = _f32(inputs["b_hh"])
    W_att = _f32(inputs["W_att"])          # [H, H]
    b_att = _f32(inputs["b_att"])          # [H]
    W_fc = _f32(inputs["W_fc"])            # [OUT, 2H]
    b_fc = _f32(inputs["b_fc"])            # [OUT]

    # embedding stream  [iters, E, COLS]; emb_stream[k,e,s*B+b] = X[b,32k+s,e]
    X = embed[trg[:, :t_total]]                       # [B, t, E]
    es = np.zeros((iters, E, COLS), np.float32)
    xt = X.transpose(2, 1, 0)                         # [E, t, B]
    es[:nch] = (
        xt.reshape(E, nch, CH, B).transpose(1, 0, 2, 3).reshape(nch, E, COLS)
    )
    es = _bf(es)

    # per-cell weights (permuted gate rows)
    cells = [(0, 0), (0, 1), (1, 0), (1, 1)]          # (layer, dir)
    zeros_emb = _bf(np.zeros((E, 4 * H)))
    zeros_in = _bf(np.zeros((2 * H, 4 * H)))

    valid_f = (np.arange(t_total)[None, :] < trg_len[:, None]).astype(np.float32)  # [B,t]
    mask_f = np.where(np.arange(S)[None, :] < trg_len[:, None], 0.0, -1e30).astype(np.float32)

    encT = enc.transpose(0, 2, 1)                     # [B, H, S]

    in_maps = []
    for c in range(N_CORES):
        cell = c % 4
        layer, d = cells[cell]
        if layer == 0:
            wemb = _bf(W_ih0[d][perm].T)              # [E, 4H]
            win = zeros_in
        else:
            wemb = zeros_emb
            win = _bf(W_ih1[d][perm].T)               # [2H, 4H]
        whh = _bf(W_hh[layer, d][perm].T)             # [H, 4H]
        bp = (b_ih[layer, d] + b_hh[layer, d])[perm]  # [4H]
        biasp = _f32(bp.reshape(MC, 128).T)           # [128, MC]
        hin = h0[layer, d]                            # [B, H]
        cin = c0[layer, d]
        h_init = _bf(hin.T.reshape(HC, 128, B).transpose(1, 0, 2))   # [128,HC,B]
        c_init = _f32(cin.T.reshape(HC, 128, B).transpose(1, 0, 2))
        a = 1.0 if layer == 0 else 0.0
        alpha = _f32(np.full((128, 1), a))
        beta = _f32(np.full((128, 1), 1.0 - a))

        bidx = list(range(B))

        m = dict(
            w_emb=wemb, w_in=win, w_hh=whh, biasp=biasp,
            h_init=h_init, c_init=c_init, alpha=alpha, beta=beta,
            emb_stream=es,
            enc_lhsT=_bf(enc[bidx]),                  # [2, S, H]
            encT_rhs=_bf(encT[bidx]),                 # [2, H, S]
            w_attT=_bf(W_att.T),
            b_att_in=_f32(b_att.reshape(HC, 128).T),
            mask_row=_bf(mask_f[bidx][:, None, :]),   # [B,1,S]
            valid_in=_f32(
                valid_f[bidx].reshape(B, tc_n, 128).transpose(0, 2, 1)
            ),
            w_fcT=_bf(W_fc.T),                        # [2H, OUT]
            b_fc_row=_bf(b_fc[None, :]),
        )
        in_maps.append(m)
    return in_maps


_NC_CACHE = {}


def kernel(**inputs) -> np.ndarray:
    nch = int(os.environ.get("KERNEL_NCH", NCH))
    if nch not in _NC_CACHE:
        _NC_CACHE[nch] = build_nc(nch)
    nc = _NC_CACHE[nch]
    in_maps = _prep_inputs(inputs, nch)
    r = run_bass_kernel_spmd(nc, in_maps, list(range(N_CORES)))
    return np.asarray(r.results[0]["out"], np.float32)


# revision 12
# speedup vs baseline: 7.5018x; 7.5018x over previous
"""Bass/Trainium2 kernel for nn_Decoder (2-layer bidir-style LSTM decoder
with general attention + fc), distributed over 8 NeuronCores.

Architecture (SPMD, one uniform program; per-core behavior differs only in
input DATA):
  - 4 LSTM cells (L0f, L0b, L1f, L1b) -> cores 0..3; cores 4..7 mirror 0..3.
  - Sequential scan is chunked: CH=32 timesteps per chunk. One AllGather per
    chunk exchanges each cell's h-chunk. Layer 1 runs one chunk behind
    layer 0 (its input-to-hidden term consumes L0's h from the previous AG).
  - Per-chunk input-to-hidden precompute: ih = W_emb @ emb_chunk +
    W_in @ [h0f; h0b]_chunk (+bias); per-core zero weights make the same
    program compute the right thing for every cell.
  - Per step: gates = ih[:, t] + Whh @ h_{t-1} via PE (weights stationary,
    gates on partitions, batch=16 moving), then an 8-op ACT/DVE chain for
    the LSTM nonlinearity. Gate order is permuted to i,f,o,g on host so one
    Sigmoid covers i,f,o and one Tanh covers g.
  - dec_t = h1f + h1b accumulated from AG slots 2,3 into per-core DRAM.
  - Post phase (after scan): per-core, its 2 batch elements: enc_proj,
    masked softmax attention, context, fc, all as dense bf16 matmuls.

Numerics: bf16 weights/activations with fp32 PSUM accumulation and fp32
cell state c. Expected rel err ~1e-3..1e-2 vs the fp32 reference.
"""

import os
import sys

sys.path.insert(0, "/opt/trn_rl_repo")

import numpy as np
import ml_dtypes

import concourse.bass as bass
import concourse.mybir as mybir
import concourse.tile as tile
from concourse import bacc
from concourse.bass_utils import run_bass_kernel_spmd

# ---- problem constants (hardcoded per contract) ----
L = 2
H = 512
E = 512
B = 16
T = 512
S = 512
VOCAB = 1001
OUT = 1000

N_CORES = 8
CH = 32                       # timesteps per chunk
NCH = T // CH                 # 16 chunks
ITERS = NCH + 1               # L1 lags one chunk
COLS = CH * B                 # 512 columns per chunk (s-major, b-minor)
HC = H // 128                 # 4 H-chunks
MC = (4 * H) // 128           # 16 gate M-chunks
TC = T // 128                 # 4 T-chunks (post phase)
SC = S // 128                 # 4 S-chunks
BF = mybir.dt.bfloat16
F32 = mybir.dt.float32
I32 = mybir.dt.int32

# gate permutation: torch order i,f,g,o -> i,f,o,g  (rows of the 4H dim)
def _gate_perm():
    idx = np.arange(4 * H)
    return np.concatenate([idx[0:H], idx[H:2 * H], idx[3 * H:4 * H], idx[2 * H:3 * H]])


def _bf(x):
    return np.ascontiguousarray(np.asarray(x, dtype=np.float32)).astype(ml_dtypes.bfloat16)


def _f32(x):
    return np.ascontiguousarray(np.asarray(x, dtype=np.float32))


def build_nc(nch=NCH):
    iters = nch + 1
    t_total = nch * CH
    tc_n = t_total // 128
    nc = bacc.Bacc("TRN2", target_bir_lowering=False, debug=False, num_devices=N_CORES)

    # ---- DRAM inputs ----
    w_emb = nc.dram_tensor("w_emb", [E, 4 * H], BF, kind="ExternalInput")
    w_in = nc.dram_tensor("w_in", [2 * H, 4 * H], BF, kind="ExternalInput")
    w_hh = nc.dram_tensor("w_hh", [H, 4 * H], BF, kind="ExternalInput")
    biasp = nc.dram_tensor("biasp", [128, MC], F32, kind="ExternalInput")
    h_init = nc.dram_tensor("h_init", [128, HC, B], BF, kind="ExternalInput")
    c_init = nc.dram_tensor("c_init", [128, HC, B], F32, kind="ExternalInput")
    alpha = nc.dram_tensor("alpha", [128, 1], F32, kind="ExternalInput")
    beta = nc.dram_tensor("beta", [128, 1], F32, kind="ExternalInput")
    emb_stream = nc.dram_tensor("emb_stream", [iters, E, COLS], BF, kind="ExternalInput")
    enc_lhsT = nc.dram_tensor("enc_lhsT", [B, S, H], BF, kind="ExternalInput")
    encT_rhs = nc.dram_tensor("encT_rhs", [B, H, S], BF, kind="ExternalInput")
    w_attT = nc.dram_tensor("w_attT", [E, H], BF, kind="ExternalInput")
    b_att_in = nc.dram_tensor("b_att_in", [128, HC], F32, kind="ExternalInput")
    mask_row = nc.dram_tensor("mask_row", [B, 1, S], BF, kind="ExternalInput")
    valid_in = nc.dram_tensor("valid_in", [B, CH, nch], F32, kind="ExternalInput")
    w_fcT = nc.dram_tensor("w_fcT", [2 * H, OUT], BF, kind="ExternalInput")
    b_fc_row = nc.dram_tensor("b_fc_row", [1, OUT], BF, kind="ExternalInput")
    out_d = nc.dram_tensor("out", [B, t_total, OUT], F32, kind="ExternalOutput")

    # ---- DRAM internals ----
    ag_out = nc.dram_tensor("ag_out", [N_CORES, H, COLS], BF, addr_space="Shared")
    dec_dram = nc.dram_tensor("dec_dram", [B, nch, 128, HC * CH], BF)

    groups = [list(range(N_CORES))]

    with tile.TileContext(nc) as tc:
        with (
            tc.tile_pool(name="wpool", bufs=1) as wpool,
            tc.tile_pool(name="spool", bufs=2) as spool,
            tc.tile_pool(name="steppool", bufs=3) as steppool,
            tc.tile_pool(name="pg", bufs=2, space="PSUM") as pg,
            tc.tile_pool(name="pih", bufs=2, space="PSUM") as pih,
            tc.tile_pool(name="ppost", bufs=2, space="PSUM") as ppost,
            tc.tile_pool(name="dpool", bufs=2, space="DRAM") as dpool,
        ):
            # ---- persistent SBUF ----
            wemb_sb = wpool.tile([128, HC, 4 * H], BF, tag="wemb")
            nc.sync.dma_start(wemb_sb[:], w_emb.rearrange("(k p) m -> p k m", p=128))
            win_sb = wpool.tile([128, 2 * HC, 4 * H], BF, tag="win")
            nc.sync.dma_start(win_sb[:], w_in.rearrange("(k p) m -> p k m", p=128))
            whh_sb = wpool.tile([128, HC, 4 * H], BF, tag="whh")
            nc.sync.dma_start(whh_sb[:], w_hh.rearrange("(k p) m -> p k m", p=128))
            biasp_sb = wpool.tile([128, MC], F32, tag="biasp")
            nc.sync.dma_start(biasp_sb[:], biasp[:])
            hinit_sb = wpool.tile([128, HC, B], BF, tag="hinit")
            nc.sync.dma_start(hinit_sb[:], h_init[:])
            cinit_sb = wpool.tile([128, HC, B], F32, tag="cinit")
            nc.sync.dma_start(cinit_sb[:], c_init[:])
            alpha_sb = wpool.tile([128, 1], F32, tag="alpha")
            nc.sync.dma_start(alpha_sb[:], alpha[:])
            beta_sb = wpool.tile([128, 1], F32, tag="beta")
            nc.sync.dma_start(beta_sb[:], beta[:])

            # zero-fill ag_in once, prologue AllGather -> ag_out defined zeros
            zer = spool.tile([128, HC, COLS], BF, tag="zer", bufs=1)
            nc.vector.memset(zer[:], 0.0)
            ag_in0 = dpool.tile([H, COLS], BF, tag="ag_in")
            nc.sync.dma_start(ag_in0.rearrange("(k p) c -> p k c", p=128), zer[:])
            nc.gpsimd.collective_compute(
                "AllGather", mybir.AluOpType.bypass, replica_groups=groups,
                ins=[ag_in0.opt()], outs=[ag_out[:].opt()],
            )

            accum_prev = None
            c_cur = None
            for k in range(iters):
                # ---------- per-iteration stream tiles ----------
                emb_t = spool.tile([128, HC, COLS], BF, tag="emb_t")
                nc.sync.dma_start(
                    emb_t[:], emb_stream[k].rearrange("(k p) c -> p k c", p=128)
                )
                xh_t = spool.tile([128, 2 * HC, COLS], BF, tag="xh_t")
                nc.sync.dma_start(
                    xh_t[:],
                    ag_out[0:2].rearrange("s (k p) c -> p (s k) c", p=128),
                )

                # ---------- ih precompute for this chunk ----------
                ih = spool.tile([128, MC, COLS], BF, tag="ih", bufs=1)
                for m in range(MC):
                    ps = pih.tile([128, COLS], F32, tag="ihps")
                    for kk in range(HC):
                        nc.tensor.matmul(
                            ps[:], wemb_sb[:, kk, m * 128:(m + 1) * 128],
                            emb_t[:, kk, :], start=(kk == 0), stop=False,
                        )
                    for kk in range(2 * HC):
                        nc.tensor.matmul(
                            ps[:], win_sb[:, kk, m * 128:(m + 1) * 128],
                            xh_t[:, kk, :], start=False, stop=(kk == 2 * HC - 1),
                        )
                    nc.vector.tensor_scalar_add(ih[:, m, :], ps[:], biasp_sb[:, m:m + 1])

                # ---------- state carry / blend ----------
                accum = spool.tile([128, HC, (CH + 1) * B], BF, tag="accum")
                if k == 0:
                    nc.vector.tensor_copy(accum[:, :, 0:B], hinit_sb[:])
                    c_new0 = steppool.tile([128, HC, B], F32, tag="c")
                    nc.vector.tensor_copy(c_new0[:], cinit_sb[:])
                    c_cur = c_new0
                elif k == 1:
                    # L1 cores reset to init (alpha=0,beta=1); L0 keep (1,0)
                    t1 = steppool.tile([128, HC, B], F32, tag="blend")
                    nc.vector.tensor_scalar_mul(t1[:], accum_prev[:, :, CH * B:], alpha_sb[:, 0:1])
                    t2 = steppool.tile([128, HC, B], F32, tag="blend")
                    nc.vector.tensor_scalar_mul(t2[:], hinit_sb[:], beta_sb[:, 0:1])
                    nc.vector.tensor_add(accum[:, :, 0:B], t1[:], t2[:])
                    t3 = steppool.tile([128, HC, B], F32, tag="blend")
                    nc.vector.tensor_scalar_mul(t3[:], c_cur[:], alpha_sb[:, 0:1])
                    t4 = steppool.tile([128, HC, B], F32, tag="blend")
                    nc.vector.tensor_scalar_mul(t4[:], cinit_sb[:], beta_sb[:, 0:1])
                    c_new1 = steppool.tile([128, HC, B], F32, tag="c")
                    nc.vector.tensor_add(c_new1[:], t3[:], t4[:])
                    c_cur = c_new1
                else:
                    nc.vector.tensor_copy(accum[:, :, 0:B], accum_prev[:, :, CH * B:])

                # ---------- CH recurrence steps ----------
                for s in range(CH):
                    g_ps = pg.tile([128, MC, B], F32, tag="g")
                    for m in range(MC):
                        for kk in range(HC):
                            nc.tensor.matmul(
                                g_ps[:, m, :],
                                whh_sb[:, kk, m * 128:(m + 1) * 128],
                                accum[:, kk, s * B:(s + 1) * B],
                                start=(kk == 0), stop=(kk == HC - 1),
                            )
                    g_sb = steppool.tile([128, MC, B], F32, tag="g_sb")
                    nc.vector.tensor_add(g_sb[:], g_ps[:], ih[:, :, s * B:(s + 1) * B])
                    sig = steppool.tile([128, 12, B], F32, tag="sig")
                    nc.scalar.activation(sig[:], g_sb[:, 0:12, :], mybir.ActivationFunctionType.Sigmoid)
                    tg = steppool.tile([128, HC, B], F32, tag="tg")
                    nc.scalar.activation(tg[:], g_sb[:, 12:16, :], mybir.ActivationFunctionType.Tanh)
                    m1 = steppool.tile([128, HC, B], F32, tag="m1")
                    nc.vector.tensor_mul(m1[:], sig[:, 4:8, :], c_cur[:])
                    m2 = steppool.tile([128, HC, B], F32, tag="m2")
                    nc.vector.tensor_mul(m2[:], sig[:, 0:4, :], tg[:])
                    c_new = steppool.tile([128, HC, B], F32, tag="c")
                    nc.vector.tensor_add(c_new[:], m1[:], m2[:])
                    tc_t = steppool.tile([128, HC, B], F32, tag="tc")
                    nc.scalar.activation(tc_t[:], c_new[:], mybir.ActivationFunctionType.Tanh)
                    nc.vector.tensor_mul(accum[:, :, (s + 1) * B:(s + 2) * B], sig[:, 8:12, :], tc_t[:])
                    c_cur = c_new

                # ---------- exchange ----------
                ag_in = dpool.tile([H, COLS], BF, tag="ag_in")
                nc.sync.dma_start(
                    ag_in.rearrange("(k p) c -> p k c", p=128), accum[:, :, B:]
                )
                nc.gpsimd.collective_compute(
                    "AllGather", mybir.AluOpType.bypass, replica_groups=groups,
                    ins=[ag_in.opt()], outs=[ag_out[:].opt()],
                )

                # ---------- dec extraction (chunk k-1) ----------
                if k >= 1:
                    s23 = spool.tile([128, 2, HC, COLS], BF, tag="s23", bufs=1)
                    nc.sync.dma_start(
                        s23[:], ag_out[2:4].rearrange("s (k p) c -> p s k c", p=128)
                    )
                    dsum = spool.tile([128, HC, CH, B], BF, tag="dsum", bufs=1)
                    nc.vector.tensor_add(
                        dsum[:],
                        s23[:, 0].rearrange("p k (s b) -> p k s b", b=B),
                        s23[:, 1].rearrange("p k (s b) -> p k s b", b=B),
                    )
                    for lb in range(B):
                        dslice = spool.tile([128, HC, CH], BF, tag="dslice", bufs=2)
                        nc.vector.tensor_copy(dslice[:], dsum[:, :, :, lb])
                        nc.sync.dma_start(
                            dec_dram[lb, k - 1],
                            dslice.rearrange("p a b -> p (a b)"),
                        )
                accum_prev = accum

            # ================= post phase =================
            identity = wpool.tile([128, 128], BF, tag="ident")
            from concourse.masks import make_identity
            make_identity(nc, identity[:])
            ones1 = wpool.tile([1, 128], BF, tag="ones1")
            nc.vector.memset(ones1[:], 1.0)

            wattT_sb = wpool.tile([128, HC, H], BF, tag="wattT")
            nc.sync.dma_start(wattT_sb[:], w_attT.rearrange("(k p) m -> p k m", p=128))
            batt_sb = wpool.tile([128, HC], F32, tag="batt")
            nc.sync.dma_start(batt_sb[:], b_att_in[:])
            wfc_sb = wpool.tile([128, 2 * HC, OUT], BF, tag="wfc")
            nc.sync.dma_start(wfc_sb[:], w_fcT.rearrange("(k p) m -> p k m", p=128))
            bfc_sb = wpool.tile([1, OUT], BF, tag="bfc")
            nc.sync.dma_start(bfc_sb[:], b_fc_row[:])

            for lb in range(B):
                encT_sb = spool.tile([128, HC, S], BF, tag="encT", bufs=1)
                nc.sync.dma_start(
                    encT_sb[:], encT_rhs[lb].rearrange("(k p) s -> p k s", p=128)
                )
                enc_sb = spool.tile([128, SC, H], BF, tag="enc", bufs=1)
                nc.sync.dma_start(
                    enc_sb[:], enc_lhsT[lb].rearrange("(k p) h -> p k h", p=128)
                )
                mask_sb = spool.tile([1, S], BF, tag="mask", bufs=1)
                nc.sync.dma_start(mask_sb[:], mask_row[lb])
                valid_sb = spool.tile([CH, nch], F32, tag="valid", bufs=1)
                nc.sync.dma_start(valid_sb[:], valid_in[lb])
                dec_sb = spool.tile([128, nch, HC, CH], BF, tag="dec_sb", bufs=1)
                nc.sync.dma_start(
                    dec_sb.rearrange("p n a b -> p n (a b)"),
                    dec_dram[lb].rearrange("n p x -> p n x"),
                )

                # enc_projT [H, S]
                epT = spool.tile([128, HC, S], BF, tag="epT", bufs=1)
                for m in range(HC):
                    pp = ppost.tile([128, S], F32, tag="pp")
                    for e in range(HC):
                        nc.tensor.matmul(
                            pp[:], wattT_sb[:, e, m * 128:(m + 1) * 128],
                            encT_sb[:, e, :], start=(e == 0), stop=(e == HC - 1),
                        )
                    nc.vector.tensor_scalar_add(epT[:, m, :], pp[:], batt_sb[:, m:m + 1])

                attT = spool.tile([128, SC, t_total], BF, tag="attT", bufs=1)
                for t in range(nch):
                    sp = ppost.tile([CH, S], F32, tag="pp")
                    for hk in range(HC):
                        nc.tensor.matmul(
                            sp[:], dec_sb[:, t, hk, :],
                            epT[:, hk, :], start=(hk == 0), stop=False,
                        )
                    nc.tensor.matmul(
                        sp[:], ones1[:, 0:CH], mask_sb[:], start=False, stop=True,
                    )
                    mx = steppool.tile([CH, 1], F32, tag="mx")
                    nc.vector.reduce_max(mx[:], sp[:], axis=mybir.AxisListType.X)
                    negmax = steppool.tile([CH, 1], F32, tag="negmax")
                    nc.scalar.mul(negmax[:], mx[:], -1.0)
                    att = steppool.tile([CH, S], BF, tag="att")
                    sumexp = steppool.tile([CH, 1], F32, tag="sumexp")
                    nc.scalar.activation(
                        att[:], sp[:], mybir.ActivationFunctionType.Exp,
                        bias=negmax[:], accum_out=sumexp[:],
                    )
                    recip = steppool.tile([CH, 1], F32, tag="recip")
                    nc.vector.reciprocal(recip[:], sumexp[:])
                    attn = steppool.tile([CH, S], BF, tag="attn")
                    nc.vector.tensor_scalar_mul(attn[:], att[:], recip[:])
                    for sk in range(SC):
                        tp = ppost.tile([128, CH], BF, tag="ptr")
                        nc.tensor.transpose(tp[:], attn[:, sk * 128:(sk + 1) * 128], identity[0:CH, 0:CH])
                        nc.vector.tensor_copy(attT[:, sk, t * CH:(t + 1) * CH], tp[:])

                ctxT = spool.tile([128, HC, t_total], BF, tag="ctxT", bufs=1)
                for m in range(HC):
                    cp = ppost.tile([128, t_total], F32, tag="pp")
                    for sk in range(SC):
                        nc.tensor.matmul(
                            cp[:], enc_sb[:, sk, m * 128:(m + 1) * 128],
                            attT[:, sk, :], start=(sk == 0), stop=(sk == SC - 1),
                        )
                    nc.vector.tensor_copy(ctxT[:, m, :], cp[:])

                for t in range(nch):
                    for nhalf in range(2):
                        ncols = OUT // 2
                        fp = ppost.tile([CH, ncols], F32, tag="pp")
                        for kk in range(2 * HC):
                            lhs = (dec_sb[:, t, kk, :] if kk < HC
                                   else ctxT[:, kk - HC, t * CH:(t + 1) * CH])
                            nc.tensor.matmul(
                                fp[:], lhs, wfc_sb[:, kk, nhalf * ncols:(nhalf + 1) * ncols],
                                start=(kk == 0), stop=False,
                            )
                        nc.tensor.matmul(
                            fp[:], ones1[:, 0:CH], bfc_sb[:, nhalf * ncols:(nhalf + 1) * ncols],
                            start=False, stop=True,
                        )
                        osb = steppool.tile([CH, ncols], F32, tag="osb")
                        nc.vector.tensor_scalar_mul(osb[:], fp[:], valid_sb[:, t:t + 1])
                        nc.sync.dma_start(
                            out_d[lb, t * CH:(t + 1) * CH, nhalf * ncols:(nhalf + 1) * ncols],
                            osb[:],
                        )

    nc.compile()
    return nc


# ---------------- host-side preparation ----------------

def _prep_inputs(inputs, nch=NCH):
    iters = nch + 1
    t_total = nch * CH
    nch_ = nch
    tc_n = t_total // 128
    perm = _gate_perm()

    trg = np.asarray(inputs["trg_inputs"]).astype(np.int64)
    trg_len = np.asarray(inputs["trg_len"]).astype(np.int64)
    enc = _f32(inputs["encoder_outputs"])
    h0 = _f32(inputs["h0"]).reshape(L, 2, B, H)
    c0 = _f32(inputs["c0"]).reshape(L, 2, B, H)
    embed = _f32(inputs["embed"])
    W_ih0 = _f32(inputs["W_ih0"])          # [2, 4H, E]
    W_ih1 = _f32(inputs["W_ih1"])[0]       # [2, 4H, 2H]
    W_hh = _f32(inputs["W_hh"])            # [L, 2, 4H, H]
    b_ih = _f32(inputs["b_ih"])            # [L, 2, 4H]
    b_hh = _f32(inputs["b_hh"])
    W_att = _f32(inputs["W_att"])          # [H, H]
    b_att = _f32(inputs["b_att"])          # [H]
    W_fc = _f32(inputs["W_fc"])            # [OUT, 2H]
    b_fc = _f32(inputs["b_fc"])            # [OUT]

    # embedding stream  [iters, E, COLS]; emb_stream[k,e,s*B+b] = X[b,32k+s,e]
    X = embed[trg[:, :t_total]]                       # [B, t, E]
    es = np.zeros((iters, E, COLS), np.float32)
    xt = X.transpose(2, 1, 0)                         # [E, t, B]
    es[:nch] = (
        xt.reshape(E, nch, CH, B).transpose(1, 0, 2, 3).reshape(nch, E, COLS)
    )
    es = _bf(es)

    # per-cell weights (permuted gate rows)
    cells = [(0, 0), (0, 1), (1, 0), (1, 1)]          # (layer, dir)
    zeros_emb = _bf(np.zeros((E, 4 * H)))
    zeros_in = _bf(np.zeros((2 * H, 4 * H)))

    valid_f = (np.arange(t_total)[None, :] < trg_len[:, None]).astype(np.float32)  # [B,t]
    mask_f = np.where(np.arange(S)[None, :] < trg_len[:, None], 0.0, -1e30).astype(np.float32)

    encT = enc.transpose(0, 2, 1)                     # [B, H, S]

    in_maps = []
    for c in range(N_CORES):
        cell = c % 4
        layer, d = cells[cell]
        if layer == 0:
            wemb = _bf(W_ih0[d][perm].T)              # [E, 4H]
            win = zeros_in
        else:
            wemb = zeros_emb
            win = _bf(W_ih1[d][perm].T)               # [2H, 4H]
        whh = _bf(W_hh[layer, d][perm].T)             # [H, 4H]
        bp = (b_ih[layer, d] + b_hh[layer, d])[perm]  # [4H]
        biasp = _f32(bp.reshape(MC, 128).T)           # [128, MC]
        hin = h0[layer, d]                            # [B, H]
        cin = c0[layer, d]
        h_init = _bf(hin.T.reshape(HC, 128, B).transpose(1, 0, 2))   # [128,HC,B]
        c_init = _f32(cin.T.reshape(HC, 128, B).transpose(1, 0, 2))
        a = 1.0 if layer == 0 else 0.0
        alpha = _f32(np.full((128, 1), a))
        beta = _f32(np.full((128, 1), 1.0 - a))

        bidx = list(range(B))

        m = dict(
            w_emb=wemb, w_in=win, w_hh=whh, biasp=biasp,
            h_init=h_init, c_init=c_init, alpha=alpha, beta=beta,
            emb_stream=es,
            enc_lhsT=_bf(enc[bidx]),                  # [2, S, H]
            encT_rhs=_bf(encT[bidx]),                 # [2, H, S]
            w_attT=_bf(W_att.T),
            b_att_in=_f32(b_att.reshape(HC, 128).T),
            mask_row=_bf(mask_f[bidx][:, None, :]),   # [B,1,S]
            valid_in=_f32(
                valid_f[bidx].reshape(B, nch_, CH).transpose(0, 2, 1)
            ),
            w_fcT=_bf(W_fc.T),                        # [2H, OUT]
            b_fc_row=_bf(b_fc[None, :]),
        )
        in_maps.append(m)
    return in_maps


_NC_CACHE = {}


def kernel(**inputs) -> np.ndarray:
    nch = int(os.environ.get("KERNEL_NCH", NCH))
    if nch not in _NC_CACHE:
        _NC_CACHE[nch] = build_nc(nch)
    nc = _NC_CACHE[nch]
    in_maps = _prep_inputs(inputs, nch)
    r = run_bass_kernel_spmd(nc, in_maps, list(range(N_CORES)))
    return np.asarray(r.results[0]["out"], np.float32)


# revision 15
# speedup vs baseline: 7.5199x; 1.0024x over previous
"""Bass/Trainium2 kernel for nn_Decoder (2-layer bidir-style LSTM decoder
with general attention + fc), distributed over 8 NeuronCores.

Architecture (SPMD, one uniform program; per-core behavior differs only in
input DATA):
  - 4 LSTM cells (L0f, L0b, L1f, L1b) -> cores 0..3; cores 4..7 mirror 0..3.
  - Sequential scan is chunked: CH=32 timesteps per chunk. One AllGather per
    chunk exchanges each cell's h-chunk. Layer 1 runs one chunk behind
    layer 0 (its input-to-hidden term consumes L0's h from the previous AG).
  - Per-chunk input-to-hidden precompute: ih = W_emb @ emb_chunk +
    W_in @ [h0f; h0b]_chunk (+bias); per-core zero weights make the same
    program compute the right thing for every cell.
  - Per step: gates = ih[:, t] + Whh @ h_{t-1} via PE (weights stationary,
    gates on partitions, batch=16 moving), then an 8-op ACT/DVE chain for
    the LSTM nonlinearity. Gate order is permuted to i,f,o,g on host so one
    Sigmoid covers i,f,o and one Tanh covers g.
  - dec_t = h1f + h1b accumulated from AG slots 2,3 into per-core DRAM.
  - Post phase (after scan): per-core, its 2 batch elements: enc_proj,
    masked softmax attention, context, fc, all as dense bf16 matmuls.

Numerics: bf16 weights/activations with fp32 PSUM accumulation and fp32
cell state c. Expected rel err ~1e-3..1e-2 vs the fp32 reference.
"""

import os
import sys

sys.path.insert(0, "/opt/trn_rl_repo")

import numpy as np
import ml_dtypes

import concourse.bass as bass
import concourse.mybir as mybir
import concourse.tile as tile
from concourse import bacc
from concourse.bass_utils import run_bass_kernel_spmd

# ---- problem constants (hardcoded per contract) ----
L = 2
H = 512
E = 512
B = 16
T = 512
S = 512
VOCAB = 1001
OUT = 1000

N_CORES = 8
CH = 32                       # timesteps per chunk
NCH = T // CH                 # 16 chunks
ITERS = NCH + 1               # L1 lags one chunk
COLS = CH * B                 # 512 columns per chunk (s-major, b-minor)
HC = H // 128                 # 4 H-chunks
MC = (4 * H) // 128           # 16 gate M-chunks
TC = T // 128                 # 4 T-chunks (post phase)
SC = S // 128                 # 4 S-chunks
BF = mybir.dt.bfloat16
F32 = mybir.dt.float32
I32 = mybir.dt.int32

# gate permutation: torch order i,f,g,o -> i,f,o,g  (rows of the 4H dim)
def _gate_perm():
    idx = np.arange(4 * H)
    return np.concatenate([idx[0:H], idx[H:2 * H], idx[3 * H:4 * H], idx[2 * H:3 * H]])


def _bf(x):
    return np.ascontiguousarray(np.asarray(x, dtype=np.float32)).astype(ml_dtypes.bfloat16)


def _f32(x):
    return np.ascontiguousarray(np.asarray(x, dtype=np.float32))


def build_nc(nch=NCH):
    iters = nch + 1
    t_total = nch * CH
    tc_n = t_total // 128
    nc = bacc.Bacc("TRN2", target_bir_lowering=False, debug=False, num_devices=N_CORES)

    # ---- DRAM inputs ----
    w_emb = nc.dram_tensor("w_emb", [E, 4 * H], BF, kind="ExternalInput")
    w_in = nc.dram_tensor("w_in", [2 * H, 4 * H], BF, kind="ExternalInput")
    w_hh = nc.dram_tensor("w_hh", [H, 4 * H], BF, kind="ExternalInput")
    biasp = nc.dram_tensor("biasp", [128, MC], F32, kind="ExternalInput")
    h_init = nc.dram_tensor("h_init", [128, HC, B], BF, kind="ExternalInput")
    c_init = nc.dram_tensor("c_init", [128, HC, B], F32, kind="ExternalInput")
    alpha = nc.dram_tensor("alpha", [128, 1], F32, kind="ExternalInput")
    beta = nc.dram_tensor("beta", [128, 1], F32, kind="ExternalInput")
    emb_stream = nc.dram_tensor("emb_stream", [iters, E, COLS], BF, kind="ExternalInput")
    enc_lhsT = nc.dram_tensor("enc_lhsT", [B, S, H], BF, kind="ExternalInput")
    encT_rhs = nc.dram_tensor("encT_rhs", [B, H, S], BF, kind="ExternalInput")
    w_attT = nc.dram_tensor("w_attT", [E, H], BF, kind="ExternalInput")
    b_att_in = nc.dram_tensor("b_att_in", [128, HC], F32, kind="ExternalInput")
    mask_row = nc.dram_tensor("mask_row", [B, 1, S], BF, kind="ExternalInput")
    valid_in = nc.dram_tensor("valid_in", [B, CH, nch], F32, kind="ExternalInput")
    w_fcT = nc.dram_tensor("w_fcT", [2 * H, OUT], BF, kind="ExternalInput")
    b_fc_row = nc.dram_tensor("b_fc_row", [1, OUT], BF, kind="ExternalInput")
    out_d = nc.dram_tensor("out", [B, t_total, OUT], F32, kind="ExternalOutput")

    # ---- DRAM internals ----
    ag_out = nc.dram_tensor("ag_out", [N_CORES, H, COLS], BF, addr_space="Shared")
    dec_dram = nc.dram_tensor("dec_dram", [B, nch, 128, HC * CH], BF)

    groups = [list(range(N_CORES))]

    with tile.TileContext(nc) as tc:
        with (
            tc.tile_pool(name="wpool", bufs=1) as wpool,
            tc.tile_pool(name="spool", bufs=2) as spool,
            tc.tile_pool(name="steppool", bufs=3) as steppool,
            tc.tile_pool(name="pg", bufs=2, space="PSUM") as pg,
            tc.tile_pool(name="pih", bufs=2, space="PSUM") as pih,
            tc.tile_pool(name="ppost", bufs=2, space="PSUM") as ppost,
            tc.tile_pool(name="dpool", bufs=2, space="DRAM") as dpool,
        ):
            # ---- persistent SBUF ----
            wemb_sb = wpool.tile([128, HC, 4 * H], BF, tag="wemb")
            nc.sync.dma_start(wemb_sb[:], w_emb.rearrange("(k p) m -> p k m", p=128))
            win_sb = wpool.tile([128, 2 * HC, 4 * H], BF, tag="win")
            nc.sync.dma_start(win_sb[:], w_in.rearrange("(k p) m -> p k m", p=128))
            whh_sb = wpool.tile([128, HC, 4 * H], BF, tag="whh")
            nc.sync.dma_start(whh_sb[:], w_hh.rearrange("(k p) m -> p k m", p=128))
            biasp_sb = wpool.tile([128, MC], F32, tag="biasp")
            nc.sync.dma_start(biasp_sb[:], biasp[:])
            hinit_sb = wpool.tile([128, HC, B], BF, tag="hinit")
            nc.sync.dma_start(hinit_sb[:], h_init[:])
            cinit_sb = wpool.tile([128, HC, B], F32, tag="cinit")
            nc.sync.dma_start(cinit_sb[:], c_init[:])
            alpha_sb = wpool.tile([128, 1], F32, tag="alpha")
            nc.sync.dma_start(alpha_sb[:], alpha[:])
            beta_sb = wpool.tile([128, 1], F32, tag="beta")
            nc.sync.dma_start(beta_sb[:], beta[:])

            # zero-fill ag_in once, prologue AllGather -> ag_out defined zeros
            zer = spool.tile([128, HC, COLS], BF, tag="zer", bufs=1)
            nc.vector.memset(zer[:], 0.0)
            ag_in0 = dpool.tile([H, COLS], BF, tag="ag_in")
            nc.sync.dma_start(ag_in0.rearrange("(k p) c -> p k c", p=128), zer[:])
            nc.gpsimd.collective_compute(
                "AllGather", mybir.AluOpType.bypass, replica_groups=groups,
                ins=[ag_in0.opt()], outs=[ag_out[:].opt()],
            )

            accum_prev = None
            c_cur = None
            for k in range(iters):
                # ---------- per-iteration stream tiles ----------
                emb_t = spool.tile([128, HC, COLS], BF, tag="emb_t")
                nc.sync.dma_start(
                    emb_t[:], emb_stream[k].rearrange("(k p) c -> p k c", p=128)
                )
                xh_t = spool.tile([128, 2 * HC, COLS], BF, tag="xh_t")
                nc.sync.dma_start(
                    xh_t[:],
                    ag_out[0:2].rearrange("s (k p) c -> p (s k) c", p=128),
                )

                # ---------- ih precompute for this chunk ----------
                ih = spool.tile([128, MC, COLS], BF, tag="ih", bufs=1)
                for m in range(MC):
                    ps = pih.tile([128, COLS], F32, tag="ihps")
                    for kk in range(HC):
                        nc.tensor.matmul(
                            ps[:], wemb_sb[:, kk, m * 128:(m + 1) * 128],
                            emb_t[:, kk, :], start=(kk == 0), stop=False,
                        )
                    for kk in range(2 * HC):
                        nc.tensor.matmul(
                            ps[:], win_sb[:, kk, m * 128:(m + 1) * 128],
                            xh_t[:, kk, :], start=False, stop=(kk == 2 * HC - 1),
                        )
                    nc.vector.tensor_scalar_add(ih[:, m, :], ps[:], biasp_sb[:, m:m + 1])

                # ---------- state carry / blend ----------
                accum = spool.tile([128, HC, (CH + 1) * B], BF, tag="accum")
                if k == 0:
                    nc.vector.tensor_copy(accum[:, :, 0:B], hinit_sb[:])
                    c_new0 = steppool.tile([128, HC, B], F32, tag="c")
                    nc.vector.tensor_copy(c_new0[:], cinit_sb[:])
                    c_cur = c_new0
                elif k == 1:
                    # L1 cores reset to init (alpha=0,beta=1); L0 keep (1,0)
                    t1 = steppool.tile([128, HC, B], F32, tag="blend")
                    nc.vector.tensor_scalar_mul(t1[:], accum_prev[:, :, CH * B:], alpha_sb[:, 0:1])
                    t2 = steppool.tile([128, HC, B], F32, tag="blend")
                    nc.vector.tensor_scalar_mul(t2[:], hinit_sb[:], beta_sb[:, 0:1])
                    nc.vector.tensor_add(accum[:, :, 0:B], t1[:], t2[:])
                    t3 = steppool.tile([128, HC, B], F32, tag="blend")
                    nc.vector.tensor_scalar_mul(t3[:], c_cur[:], alpha_sb[:, 0:1])
                    t4 = steppool.tile([128, HC, B], F32, tag="blend")
                    nc.vector.tensor_scalar_mul(t4[:], cinit_sb[:], beta_sb[:, 0:1])
                    c_new1 = steppool.tile([128, HC, B], F32, tag="c")
                    nc.vector.tensor_add(c_new1[:], t3[:], t4[:])
                    c_cur = c_new1
                else:
                    nc.vector.tensor_copy(accum[:, :, 0:B], accum_prev[:, :, CH * B:])

                # ---------- CH recurrence steps ----------
                for s in range(CH):
                    g_ps = pg.tile([128, MC, B], F32, tag="g")
                    for m in range(MC):
                        for kk in range(HC):
                            nc.tensor.matmul(
                                g_ps[:, m, :],
                                whh_sb[:, kk, m * 128:(m + 1) * 128],
                                accum[:, kk, s * B:(s + 1) * B],
                                start=(kk == 0), stop=(kk == HC - 1),
                            )
                    g_sb = steppool.tile([128, MC, B], F32, tag="g_sb")
                    nc.vector.tensor_add(g_sb[:], g_ps[:], ih[:, :, s * B:(s + 1) * B])
                    sig = steppool.tile([128, 12, B], F32, tag="sig")
                    nc.scalar.activation(sig[:], g_sb[:, 0:12, :], mybir.ActivationFunctionType.Sigmoid)
                    tg = steppool.tile([128, HC, B], F32, tag="tg")
                    nc.scalar.activation(tg[:], g_sb[:, 12:16, :], mybir.ActivationFunctionType.Tanh)
                    m1 = steppool.tile([128, HC, B], F32, tag="m1")
                    nc.vector.tensor_mul(m1[:], sig[:, 4:8, :], c_cur[:])
                    m2 = steppool.tile([128, HC, B], F32, tag="m2")
                    nc.vector.tensor_mul(m2[:], sig[:, 0:4, :], tg[:])
                    c_new = steppool.tile([128, HC, B], F32, tag="c")
                    nc.vector.tensor_add(c_new[:], m1[:], m2[:])
                    tc_t = steppool.tile([128, HC, B], F32, tag="tc")
                    nc.scalar.activation(tc_t[:], c_new[:], mybir.ActivationFunctionType.Tanh)
                    nc.vector.tensor_mul(accum[:, :, (s + 1) * B:(s + 2) * B], sig[:, 8:12, :], tc_t[:])
                    c_cur = c_new

                # ---------- exchange ----------
                ag_in = dpool.tile([H, COLS], BF, tag="ag_in")
                nc.sync.dma_start(
                    ag_in.rearrange("(k p) c -> p k c", p=128), accum[:, :, B:]
                )
                nc.gpsimd.collective_compute(
                    "AllGather", mybir.AluOpType.bypass, replica_groups=groups,
                    ins=[ag_in.opt()], outs=[ag_out[:].opt()],
                )

                # ---------- dec extraction (chunk k-1) ----------
                if k >= 1:
                    s23 = spool.tile([128, 2, HC, COLS], BF, tag="s23", bufs=1)
                    nc.sync.dma_start(
                        s23[:], ag_out[2:4].rearrange("s (k p) c -> p s k c", p=128)
                    )
                    dsum = spool.tile([128, HC, CH, B], BF, tag="dsum", bufs=1)
                    nc.vector.tensor_add(
                        dsum[:],
                        s23[:, 0].rearrange("p k (s b) -> p k s b", b=B),
                        s23[:, 1].rearrange("p k (s b) -> p k s b", b=B),
                    )
                    for lb in range(B):
                        dslice = spool.tile([128, HC, CH], BF, tag="dslice", bufs=2)
                        nc.vector.tensor_copy(dslice[:], dsum[:, :, :, lb])
                        nc.sync.dma_start(
                            dec_dram[lb, k - 1],
                            dslice.rearrange("p a b -> p (a b)"),
                        )
                accum_prev = accum

            # ================= post phase =================
            identity = wpool.tile([128, 128], BF, tag="ident")
            from concourse.masks import make_identity
            make_identity(nc, identity[:])
            ones1 = wpool.tile([1, 128], BF, tag="ones1")
            nc.vector.memset(ones1[:], 1.0)

            wattT_sb = wpool.tile([128, HC, H], BF, tag="wattT")
            nc.sync.dma_start(wattT_sb[:], w_attT.rearrange("(k p) m -> p k m", p=128))
            batt_sb = wpool.tile([128, HC], F32, tag="batt")
            nc.sync.dma_start(batt_sb[:], b_att_in[:])
            wfc_sb = wpool.tile([128, 2 * HC, OUT], BF, tag="wfc")
            nc.sync.dma_start(wfc_sb[:], w_fcT.rearrange("(k p) m -> p k m", p=128))
            bfc_sb = wpool.tile([1, OUT], BF, tag="bfc")
            nc.sync.dma_start(bfc_sb[:], b_fc_row[:])

            for lb in range(B):
                encT_sb = spool.tile([128, HC, S], BF, tag="encT", bufs=1)
                nc.sync.dma_start(
                    encT_sb[:], encT_rhs[lb].rearrange("(k p) s -> p k s", p=128)
                )
                enc_sb = spool.tile([128, SC, H], BF, tag="enc", bufs=1)
                nc.sync.dma_start(
                    enc_sb[:], enc_lhsT[lb].rearrange("(k p) h -> p k h", p=128)
                )
                mask_sb = spool.tile([1, S], BF, tag="mask", bufs=1)
                nc.sync.dma_start(mask_sb[:], mask_row[lb])
                valid_sb = spool.tile([CH, nch], F32, tag="valid", bufs=1)
                nc.sync.dma_start(valid_sb[:], valid_in[lb])
                dec_sb = spool.tile([128, nch, HC, CH], BF, tag="dec_sb", bufs=1)
                nc.sync.dma_start(
                    dec_sb.rearrange("p n a b -> p n (a b)"),
                    dec_dram[lb].rearrange("n p x -> p n x"),
                )

                # enc_projT [H, S]
                epT = spool.tile([128, HC, S], BF, tag="epT", bufs=1)
                for m in range(HC):
                    pp = ppost.tile([128, S], F32, tag="pp")
                    for e in range(HC):
                        nc.tensor.matmul(
                            pp[:], wattT_sb[:, e, m * 128:(m + 1) * 128],
                            encT_sb[:, e, :], start=(e == 0), stop=(e == HC - 1),
                        )
                    nc.vector.tensor_scalar_add(epT[:, m, :], pp[:], batt_sb[:, m:m + 1])

                attT = spool.tile([128, SC, t_total], BF, tag="attT", bufs=1)
                for t in range(nch):
                    sp = ppost.tile([CH, S], F32, tag="pp")
                    for hk in range(HC):
                        nc.tensor.matmul(
                            sp[:], dec_sb[:, t, hk, :],
                            epT[:, hk, :], start=(hk == 0), stop=False,
                        )
                    nc.tensor.matmul(
                        sp[:], ones1[:, 0:CH], mask_sb[:], start=False, stop=True,
                    )
                    mx = steppool.tile([CH, 1], F32, tag="mx")
                    nc.vector.reduce_max(mx[:], sp[:], axis=mybir.AxisListType.X)
                    negmax = steppool.tile([CH, 1], F32, tag="negmax")
                    nc.scalar.mul(negmax[:], mx[:], -1.0)
                    att = steppool.tile([CH, S], BF, tag="att")
                    sumexp = steppool.tile([CH, 1], F32, tag="sumexp")
                    nc.scalar.activation(
                        att[:], sp[:], mybir.ActivationFunctionType.Exp,
                        bias=negmax[:], accum_out=sumexp[:],
                    )
                    recip = steppool.tile([CH, 1], F32, tag="recip")
                    nc.vector.reciprocal(recip[:], sumexp[:])
                    attn = steppool.tile([CH, S], BF, tag="attn")
                    nc.vector.tensor_scalar_mul(attn[:], att[:], recip[:])
                    for sk in range(SC):
                        tp = ppost.tile([128, CH], BF, tag="ptr")
                        nc.tensor.transpose(tp[:], attn[:, sk * 128:(sk + 1) * 128], identity[0:CH, 0:CH])
                        nc.vector.tensor_copy(attT[:, sk, t * CH:(t + 1) * CH], tp[:])

                ctxT = spool.tile([128, HC, t_total], BF, tag="ctxT", bufs=1)
                for m in range(HC):
                    cp = ppost.tile([128, t_total], F32, tag="pp")
                    for sk in range(SC):
                        nc.tensor.matmul(
                            cp[:], enc_sb[:, sk, m * 128:(m + 1) * 128],
                            attT[:, sk, :], start=(sk == 0), stop=(sk == SC - 1),
                        )
                    nc.vector.tensor_copy(ctxT[:, m, :], cp[:])

                for t in range(nch):
                    for nhalf in range(2):
                        ncols = OUT // 2
                        fp = ppost.tile([CH, ncols], F32, tag="pp")
                        for kk in range(2 * HC):
                            lhs = (dec_sb[:, t, kk, :] if kk < HC
                                   else ctxT[:, kk - HC, t * CH:(t + 1) * CH])
                            nc.tensor.matmul(
                                fp[:], lhs, wfc_sb[:, kk, nhalf * ncols:(nhalf + 1) * ncols],
                                start=(kk == 0), stop=False,
                            )
                        nc.tensor.matmul(
                            fp[:], ones1[:, 0:CH], bfc_sb[:, nhalf * ncols:(nhalf + 1) * ncols],
                            start=False, stop=True,
                        )
                        osb = steppool.tile([CH, ncols], F32, tag="osb")
                        nc.vector.tensor_scalar_mul(osb[:], fp[:], valid_sb[:, t:t + 1])
                        nc.sync.dma_start(
                            out_d[lb, t * CH:(t + 1) * CH, nhalf * ncols:(nhalf + 1) * ncols],
                            osb[:],
                        )

    nc.compile()
    return nc


# ---------------- host-side preparation ----------------

def _prep_inputs(inputs, nch=NCH):
    iters = nch + 1
    t_total = nch * CH
    nch_ = nch
    tc_n = t_total // 128
    perm = _gate_perm()

    trg = np.asarray(inputs["trg_inputs"]).astype(np.int64)
    trg_len = np.asarray(inputs["trg_len"]).astype(np.int64)
    enc = _f32(inputs["encoder_outputs"])
    h0 = _f32(inputs["h0"]).reshape(L, 2, B, H)
    c0 = _f32(inputs["c0"]).reshape(L, 2, B, H)
    embed = _f32(inputs["embed"])
    W_ih0 = _f32(inputs["W_ih0"])          # [2, 4H, E]
    W_ih1 = _f32(inputs["W_ih1"])[0]       # [2, 4H, 2H]
    W_hh = _f32(inputs["W_hh"])            # [L, 2, 4H, H]
    b_ih = _f32(inputs["b_ih"])            # [L, 2, 4H]
    b_hh = _f32(inputs["b_hh"])
    W_att = _f32(inputs["W_att"])          # [H, H]
    b_att = _f32(inputs["b_att"])          # [H]
    W_fc = _f32(inputs["W_fc"])            # [OUT, 2H]
    b_fc = _f32(inputs["b_fc"])            # [OUT]

    # embedding stream  [iters, E, COLS]; emb_stream[k,e,s*B+b] = X[b,32k+s,e]
    X = embed[trg[:, :t_total]]                       # [B, t, E]
    es = np.zeros((iters, E, COLS), np.float32)
    xt = X.transpose(2, 1, 0)                         # [E, t, B]
    es[:nch] = (
        xt.reshape(E, nch, CH, B).transpose(1, 0, 2, 3).reshape(nch, E, COLS)
    )
    es = _bf(es)

    # per-cell weights (permuted gate rows)
    cells = [(0, 0), (0, 1), (1, 0), (1, 1)]          # (layer, dir)
    zeros_emb = _bf(np.zeros((E, 4 * H)))
    zeros_in = _bf(np.zeros((2 * H, 4 * H)))

    valid_f = (np.arange(t_total)[None, :] < trg_len[:, None]).astype(np.float32)  # [B,t]
    mask_f = np.where(np.arange(S)[None, :] < trg_len[:, None], 0.0, -1e30).astype(np.float32)

    encT = enc.transpose(0, 2, 1)                     # [B, H, S]

    in_maps = []
    for c in range(N_CORES):
        cell = c % 4
        layer, d = cells[cell]
        if layer == 0:
            wemb = _bf(W_ih0[d][perm].T)              # [E, 4H]
            win = zeros_in
        else:
            wemb = zeros_emb
            win = _bf(W_ih1[d][perm].T)               # [2H, 4H]
        whh = _bf(W_hh[layer, d][perm].T)             # [H, 4H]
        bp = (b_ih[layer, d] + b_hh[layer, d])[perm]  # [4H]
        biasp = _f32(bp.reshape(MC, 128).T)           # [128, MC]
        hin = h0[layer, d]                            # [B, H]
        cin = c0[layer, d]
        h_init = _bf(hin.T.reshape(HC, 128, B).transpose(1, 0, 2))   # [128,HC,B]
        c_init = _f32(cin.T.reshape(HC, 128, B).transpose(1, 0, 2))
        a = 1.0 if layer == 0 else 0.0
        alpha = _f32(np.full((128, 1), a))
        beta = _f32(np.full((128, 1), 1.0 - a))

        bidx = list(range(B))

        m = dict(
            w_emb=wemb, w_in=win, w_hh=whh, biasp=biasp,
            h_init=h_init, c_init=c_init, alpha=alpha, beta=beta,
            emb_stream=es,
            enc_lhsT=_bf(enc[bidx]),                  # [2, S, H]
            encT_rhs=_bf(encT[bidx]),                 # [2, H, S]
            w_attT=_bf(W_att.T),
            b_att_in=_f32(b_att.reshape(HC, 128).T),
            mask_row=_bf(mask_f[bidx][:, None, :]),   # [B,1,S]
            valid_in=_f32(
                valid_f[bidx].reshape(B, nch_, CH).transpose(0, 2, 1)
            ),
            w_fcT=_bf(W_fc.T),                        # [2H, OUT]
            b_fc_row=_bf(b_fc[None, :]),
        )
        in_maps.append(m)
    return in_maps


_NC_CACHE = {}


def kernel(**inputs) -> np.ndarray:
    nch = int(os.environ.get("KERNEL_NCH", NCH))
    if nch not in _NC_CACHE:
        _NC_CACHE[nch] = build_nc(nch)
    nc = _NC_CACHE[nch]
    in_maps = _prep_inputs(inputs, nch)
    r = run_bass_kernel_spmd(nc, in_maps, list(range(N_CORES)))
    return np.asarray(r.results[0]["out"], np.float32)
